# revision 22
# baseline (speedup 1.0000x reference)
"""Trainium2 Bass kernel for ExhaustiveBiaffineNERDecoder.

reference semantics:
  masked BatchNorm(features) -> FFN(768->4096) + ReLU
  -> reshape [B,T,16,128,2] -> start/end features
  -> scores[b,l,s,e] = sum_d start[b,s,l,d]*end[b,e,l,d] + label_bias[l]
  -> spans_mask = triu & mask & mask  (pure boolean, computed on host)

Sharding: 2-D grid over (sample-groups x label-groups), default 4x2: each core
handles 2 samples x 8 labels. This halves+quarters the replicated input
traffic vs pure data-parallel (W and features are each only partially
replicated: 9.5 MB in per core instead of 14.2 MB), with identical PE work.
BN stats are global over the batch: each core computes bn_stats over its local
samples and the per-feature (mean, E[x^2]) pairs are AllReduced across the 8
cores (6 KB on-chip collective).

Layout trick: ff_w rows are permuted on the host to [label, start/end, d_out]
order and the whole weight is transposed to [768, 4096]. The FFN then directly
produces h^T tiles [128 d_out x T tokens] per (label, start/end) -- exactly
the stationary/moving operands the biaffine matmul needs, so there are no
on-device transposes at all.

Matmuls run as float32r (full fp32 data, reduced-precision PE multiply) which
streams at bf16 rate for moving dims >= 256; measured end-to-end error vs the
fp32 reference is ~1.4e-4 scale-relative.
"""

import os

import numpy as np

import concourse.bacc as bacc
import concourse.mybir as mybir
import concourse.tile as tile
from concourse import bass_utils

F32 = mybir.dt.float32
F32R = mybir.dt.float32r
AF = mybir.ActivationFunctionType
ALU = mybir.AluOpType

B, T, D = 8, 512, 768
NL, LD = 16, 128
O = NL * LD * 2  # 4096
DC = D // 128  # 6 contraction chunks
BN_EPS = 1e-5
N_CORES = 8

_CACHE = {}
last_run_info = None  # BassKernelResults of the most recent run (for profiling)


def _shard():
    s = os.environ.get("BIAFFINE_SHARD", "4x2")
    sg, lg = (int(v) for v in s.split("x"))
    assert sg * lg == N_CORES
    return sg, lg


def _stats_mode():
    return os.environ.get("BIAFFINE_STATS", "ar")  # "ar" (AllReduce) or "local"


def _build_nc(stats_mode="ar", bench_loop=1, loop_scope="body", sg=4, lg=2, hb=4, scb=4, tb=2):
    spc = B // sg  # samples per core
    lpc = NL // lg  # labels per core
    TL = spc * T  # local tokens
    OL = lpc * LD * 2  # local FFN output cols
    NH = TL // 512  # moving-dim halves (fp32 moving max is 512)
    QW = 512 if OL <= 2048 else 1024
    NQ = OL // QW

    nc = bacc.Bacc("TRN2", target_bir_lowering=False, debug=False, num_devices=N_CORES)

    wT = nc.dram_tensor("wT", [D, OL], F32, kind="ExternalInput")
    xto = nc.dram_tensor("xto", [D, TL], F32, kind="ExternalInput")
    maskf = nc.dram_tensor("maskf", [1, TL], F32, kind="ExternalInput")
    gamma = nc.dram_tensor("gamma", [D], F32, kind="ExternalInput")
    beta = nc.dram_tensor("beta", [D], F32, kind="ExternalInput")
    ffb = nc.dram_tensor("ffb", [OL], F32, kind="ExternalInput")
    lbias = nc.dram_tensor("lbias", [1, lpc], F32, kind="ExternalInput")
    scores = nc.dram_tensor("scores", [spc, lpc, T, T], F32, kind="ExternalOutput")
    if stats_mode == "ar":
        cc_in = nc.dram_tensor("cc_in", [128, DC, 2], F32, kind="Internal")
        cc_out = nc.dram_tensor(
            "cc_out", [128, DC, 2], F32, kind="Internal", addr_space="Shared"
        )

    with tile.TileContext(nc) as tc:
        with (
            tc.tile_pool(name="const", bufs=1) as const,
            tc.tile_pool(name="wp", bufs=1) as wp,
            tc.tile_pool(name="xstat", bufs=2) as xstat,
            tc.tile_pool(name="stats", bufs=1) as stats,
            tc.tile_pool(name="xn", bufs=1) as xnp,
            tc.tile_pool(name="tmp", bufs=tb) as tmpp,
            tc.tile_pool(name="h", bufs=hb) as hp,
            tc.tile_pool(name="sc", bufs=scb) as scp,
            tc.tile_pool(name="ph", bufs=4, space="PSUM") as psum_h,
            tc.tile_pool(name="psc", bufs=4, space="PSUM") as psum_s,
        ):
            # ---- constants ----
            g_t = const.tile([128, DC], F32, tag="g")
            nc.sync.dma_start(out=g_t[:], in_=gamma[:].rearrange("(c p) -> p c", p=128))
            bt_t = const.tile([128, DC], F32, tag="bt")
            nc.sync.dma_start(out=bt_t[:], in_=beta[:].rearrange("(c p) -> p c", p=128))
            # local ff_b in [d_out, label, se] order (matches W row permutation)
            ffb_t = const.tile([128, lpc, 2], F32, tag="ffb")
            nc.sync.dma_start(
                out=ffb_t[:],
                in_=ffb[:].rearrange("(l d s) -> d l s", l=lpc, d=128, s=2),
            )
            lb_t = const.tile([128, lpc], F32, tag="lb")
            nc.sync.dma_start(out=lb_t[:], in_=lbias[:].partition_broadcast(128))
            mask_t = const.tile([128, TL], F32, tag="mask")
            nc.sync.dma_start(out=mask_t[:], in_=maskf[:].partition_broadcast(128))
            eps_t = const.tile([128, 1], F32, tag="eps")
            nc.vector.memset(eps_t[:], BN_EPS)

            # everything per-iteration lives in prefix() + _emit_main() so the
            # bench modes can wrap either just the main compute ("body") or
            # the whole pipeline ("full") in an on-device repeat loop.
            def prefix(collective_ok=True):
                # ---- local feature strips (stats AND FFN input, normalized
                # in place) ----
                xto_c = xto[:].rearrange("(c p) t -> c p t", p=128)
                xo_tiles = []
                for c in range(DC):
                    xo_t = xstat.tile([128, TL], F32, tag=f"xo{c}")
                    nc.sync.dma_start(out=xo_t[:], in_=xto_c[c])
                    xo_tiles.append(xo_t)

                # ---- weight blocks, j-interleaved column order so the FFN
                # can start as soon as the first few blocks arrive ----
                wT_c = wT[:].rearrange("(c p) o -> c p o", p=128)
                w_blocks = [[None] * NQ for _ in range(DC)]
                for q in range(NQ):
                    for c in range(DC):
                        w_b = wp.tile([128, QW], F32R, tag=f"w{c}_{q}")
                        # issue W streaming from the otherwise-idle GpSimd queue
                        # so the ~1us per-dma_start setup doesn't serialize on
                        # the sync engine with everything else
                        nc.gpsimd.dma_start(
                            out=w_b[:],
                            in_=wT_c[c][:, q * QW : (q + 1) * QW].bitcast(F32R),
                        )
                        w_blocks[c][q] = w_b

                # ---- BN statistics (partial -> AllReduce, or local-only) ----
                send = stats.tile([128, DC, 2], F32, tag="send")
                for c in range(DC):
                    mv_c = tmpp.tile([128, 2], F32, tag="mv")
                    st = tmpp.tile([128, NH, 6], F32, tag="bnst")
                    for k in range(NH):
                        nc.vector.bn_stats(
                            out=st[:, k, :], in_=xo_tiles[c][:, k * 512 : (k + 1) * 512]
                        )
                    nc.vector.bn_aggr(out=mv_c[:], in_=st[:])
                    # send[:,c,0] = mean_c ; send[:,c,1] = var_c + mean_c^2
                    nc.vector.tensor_copy(send[:, c, 0:1], mv_c[:, 0:1])
                    m2 = tmpp.tile([128, 1], F32, tag="m2")
                    nc.vector.tensor_mul(m2[:], mv_c[:, 0:1], mv_c[:, 0:1])
                    nc.vector.tensor_add(send[:, c, 1:2], mv_c[:, 1:2], m2[:])
                g_sum = stats.tile([128, DC, 2], F32, tag="gsum")
                if collective_ok:
                    nc.sync.dma_start(out=cc_in[:], in_=send[:])
                    nc.gpsimd.collective_compute(
                        "AllReduce",
                        ALU.add,
                        replica_groups=[list(range(N_CORES))],
                        ins=[cc_in[:]],
                        outs=[cc_out[:]],
                    )
                    nc.sync.dma_start(out=g_sum[:], in_=cc_out[:])
                else:
                    # timing-only stand-in (collectives can't sit in a loop)
                    nc.scalar.mul(g_sum[:], send[:], float(N_CORES))
                # mean6 / Ex2 -> var -> a,b   (vectorized over the 6 chunks)
                mean6 = stats.tile([128, DC], F32, tag="mean6")
                nc.scalar.mul(mean6[:], g_sum[:, :, 0], 1.0 / N_CORES)
                ex26 = stats.tile([128, DC], F32, tag="ex26")
                nc.scalar.mul(ex26[:], g_sum[:, :, 1], 1.0 / N_CORES)
                msq6 = tmpp.tile([128, DC], F32, tag="msq6")
                nc.vector.tensor_mul(msq6[:], mean6[:], mean6[:])
                var6 = stats.tile([128, DC], F32, tag="var6")
                nc.vector.tensor_sub(var6[:], ex26[:], msq6[:])

                # ---- fold BN into per-partition scale a / bias b ----
                sd6 = tmpp.tile([128, DC], F32, tag="sd6")
                nc.scalar.activation(
                    out=sd6[:], in_=var6[:], func=AF.Sqrt, bias=eps_t[:], scale=1.0
                )
                rq6 = tmpp.tile([128, DC], F32, tag="rq6")
                nc.vector.reciprocal(out=rq6[:], in_=sd6[:])
                a6 = stats.tile([128, DC], F32, tag="a6")
                nc.vector.tensor_mul(a6[:], rq6[:], g_t[:])
                t6 = tmpp.tile([128, DC], F32, tag="t6")
                nc.vector.tensor_mul(t6[:], mean6[:], a6[:])
                b6 = stats.tile([128, DC], F32, tag="b6")
                nc.vector.tensor_sub(b6[:], bt_t[:], t6[:])

                # ---- normalized+masked activations, f32r-typed tiles ----
                xn_tiles = []
                for c in range(DC):
                    t3 = tmpp.tile([128, TL], F32, tag="t3")
                    nc.vector.tensor_scalar(
                        t3[:],
                        xo_tiles[c][:],
                        a6[:, c : c + 1],
                        b6[:, c : c + 1],
                        ALU.mult,
                        ALU.add,
                    )
                    xn_c = xnp.tile([128, TL], F32R, tag=f"xn{c}")
                    nc.vector.tensor_tensor(xn_c[:], t3[:], mask_t[:], ALU.mult)
                    xn_tiles.append(xn_c)
                return w_blocks, xn_tiles

            def main_body(w_blocks, xn_tiles):
                _emit_main(
                    nc, w_blocks, xn_tiles, ffb_t, lb_t, hp, scp, psum_h, psum_s,
                    scores, spc, lpc, TL, NH, QW,
                )

            cok = stats_mode == "ar"
            if bench_loop > 1 and loop_scope == "full":
                with tc.For_i(0, bench_loop, 1) as _i:
                    wb, xn = prefix(collective_ok=False)
                    main_body(wb, xn)
            elif bench_loop > 1:
                wb, xn = prefix(collective_ok=cok)
                with tc.For_i(0, bench_loop, 1) as _i:
                    main_body(wb, xn)
            else:
                wb, xn = prefix(collective_ok=cok)
                main_body(wb, xn)

    nc.compile()
    return nc


def _emit_main(
    nc, w_blocks, xn_tiles, ffb_t, lb_t, hp, scp, psum_h, psum_s, scores,
    spc, lpc, TL, NH, QW,
):
    for l in range(lpc):
        h_pair = []
        for se in range(2):
            j = l * 2 + se
            q, jj = divmod(j * 128, QW)
            h_t = hp.tile([128, TL], F32R, tag="h")
            for half in range(NH):
                ph = psum_h.tile([128, 512], F32, tag="ph")
                for c in range(DC):
                    nc.tensor.matmul(
                        ph[:],
                        w_blocks[c][q][:, jj : jj + 128],
                        xn_tiles[c][:, half * 512 : (half + 1) * 512],
                        start=(c == 0),
                        stop=(c == DC - 1),
                    )
                nc.scalar.activation(
                    out=h_t[:, half * 512 : (half + 1) * 512],
                    in_=ph[:],
                    func=AF.Relu,
                    bias=ffb_t[:, l, se : se + 1],
                    scale=1.0,
                )
            h_pair.append(h_t)
        h_s, h_e = h_pair
        for b in range(spc):
            # one [128, 4, 512] staging tile per (l,b) -> a single 1MB DMA out
            sc_t = scp.tile([128, 4, T], F32, tag="sc")
            for i in range(4):
                psc = psum_s.tile([128, 512], F32, tag="psc")
                nc.tensor.matmul(
                    psc[:],
                    h_s[:, b * T + i * 128 : b * T + (i + 1) * 128],
                    h_e[:, b * T : (b + 1) * T],
                    start=True,
                    stop=True,
                )
                if (l * spc * 4 + b * 4 + i) % 2 == 0:
                    nc.scalar.add(sc_t[:, i, :], psc[:], lb_t[:, l : l + 1])
                else:
                    nc.vector.tensor_scalar_add(
                        sc_t[:, i, :], psc[:], lb_t[:, l : l + 1]
                    )
            nc.gpsimd.dma_start(
                out=scores[b, l].rearrange("(i p) e -> p i e", p=128), in_=sc_t[:]
            )


def _get_nc(stats_mode=None, bench_loop=1, loop_scope="body", sg=None, lg=None):
    if stats_mode is None:
        stats_mode = _stats_mode()
    if sg is None:
        sg, lg = _shard()
    key = ("nc", stats_mode, bench_loop, loop_scope, sg, lg)
    if key not in _CACHE:
        _CACHE[key] = _build_nc(stats_mode, bench_loop, loop_scope, sg, lg)
    return _CACHE[key]


def make_in_maps(features, mask_b, bn_gamma, bn_beta, ff_w, ff_b, label_bias, sg, lg):
    spc = B // sg
    lpc = NL // lg
    TL = spc * T
    OL = lpc * LD * 2

    xtf = np.ascontiguousarray(features.reshape(B * T, D).T)  # [768, 4096]
    wT = np.ascontiguousarray(
        ff_w.reshape(NL, LD, 2, D).transpose(3, 0, 2, 1).reshape(D, O)
    )  # [768, (l,se,d_out)]
    maskf = mask_b.astype(np.float32).reshape(B * T)

    in_maps = []
    for i in range(sg):
        for k in range(lg):
            in_maps.append(
                {
                    "wT": np.ascontiguousarray(wT[:, k * OL : (k + 1) * OL]),
                    "xto": np.ascontiguousarray(xtf[:, i * TL : (i + 1) * TL]),
                    "maskf": np.ascontiguousarray(
                        maskf[i * TL : (i + 1) * TL].reshape(1, TL)
                    ),
                    "gamma": bn_gamma,
                    "beta": bn_beta,
                    "ffb": np.ascontiguousarray(ff_b[k * OL : (k + 1) * OL]),
                    "lbias": np.ascontiguousarray(
                        label_bias[k * lpc : (k + 1) * lpc].reshape(1, lpc)
                    ),
                }
            )
    return in_maps


def kernel(features, mask, bn_gamma, bn_beta, ff_w, ff_b, label_bias):
    global last_run_info
    features = np.asarray(features, dtype=np.float32)
    mask_b = np.asarray(mask).astype(bool)
    bn_gamma = np.asarray(bn_gamma, dtype=np.float32)
    bn_beta = np.asarray(bn_beta, dtype=np.float32)
    ff_w = np.asarray(ff_w, dtype=np.float32)
    ff_b = np.asarray(ff_b, dtype=np.float32)
    label_bias = np.asarray(label_bias, dtype=np.float32)

    sg, lg = _shard()
    spc = B // sg
    lpc = NL // lg
    nc = _get_nc(_stats_mode(), sg=sg, lg=lg)
    in_maps = make_in_maps(
        features, mask_b, bn_gamma, bn_beta, ff_w, ff_b, label_bias, sg, lg
    )

    res = bass_utils.run_bass_kernel_spmd(
        nc,
        in_maps,
        core_ids=list(range(N_CORES)),
        trace=bool(os.environ.get("BIAFFINE_TRACE")),
    )
    last_run_info = res
    scores = np.empty((B, NL, T, T), dtype=np.float32)
    for i in range(sg):
        for k in range(lg):
            core = i * lg + k
            blk = res.results[core]["scores"]  # [spc, lpc, T, T]
            scores[i * spc : (i + 1) * spc, k * lpc : (k + 1) * lpc] = blk

    # span mask: pure boolean broadcast, no FLOPs
    triu = np.triu(np.ones((T, T), dtype=bool))
    spans = triu[None, None] & mask_b[:, None, :, None] & mask_b[:, None, None, :]
    spans = np.broadcast_to(spans, scores.shape)
    return scores, spans
